# revision 27
# baseline (speedup 1.0000x reference)
"""ContextualLoss forward on 8 Trainium2 NeuronCores.

Math (reference):
    mu[m]   = mean_c Y[c, m]                      (PONO over channels of Y)
    Xc = X - mu ; Yc = Y - mu                     (both centered by Y's mean)
    cos[i,j] = <Xc_i, Yc_j> / (|Xc_i| |Yc_j|)
    d = 1 - cos ; dn = d / (min_j d + 1e-3) ; w = exp((1 - dn)/0.1)
    A = w / sum_j w ; CX_b = mean_i max_j A ; loss = mean_b -log CX_b

Device-side structure (per core: one sample b, one 2048-row half):
  * Only Y is centered. <Xc_i, Yc_j> == <X_i, Yc_j> because Yc has zero
    channel-mean, so raw X feeds the matmul.
  * |Xc_i|^2 = sum X^2 - mu*(2*sum X - 256*mu) via cheap N=1 column-sum
    matmuls -- Xc is never materialized.
  * Main loop per 128-row tile: 16 bf16 matmuls -> PSUM halves [128,2048];
    one fused DVE tensor_tensor_reduce per half does the PSUM->SBUF move,
    the 1/|Yc_j| column scale (op0=divide by |Yc_j|), and the running
    row-max in a single pass. DVE runs nothing else in steady state.
  * Per-tile scalar chain (dmin -> exp scale/bias) runs on the Pool engine.
  * ScalarE does one fused Exp per tile; accum_out gives sum_j w for free.
  * max_j A = exp(0.01/(dmin+1e-3)) / sum_j w  (w monotone in d).

Sharding: core c -> sample b = c//2, row-half h = c%2 (2048 rows each).
Each core's Y is column-permuted host-side to [own-half | other-half] so the
identical SPMD program can read the X-half's means from columns [0, 2048).
Row reductions are permutation-invariant, so the permutation is harmless.
"""

import os
import sys

sys.path.insert(0, "/opt/trn_rl_repo")

import numpy as np

import concourse.bass as bass
import concourse.tile as tile
from concourse import bacc
from concourse import mybir
from concourse.bass_utils import run_bass_kernel_spmd

B = 4
C = 256
M = 4096  # 64*64 spatial positions
HALF = M // 2  # rows per core
NT = HALF // 128  # 16 i-tiles per core
N_CORES = 8
Q = 1024  # preprocessing quarter width
HB = 2048  # main-loop PSUM half width

F32 = mybir.dt.float32
BF16 = mybir.dt.bfloat16
AF = mybir.ActivationFunctionType
ALU = mybir.AluOpType

NEG_HUGE = -3.0e38

# 1: TTR divides PSUM scores by |Yc_j| directly (no reciprocal pass).
# 0: precompute 1/|Yc_j| (DVE reciprocal) and multiply in the TTR.
# NOTE: the real DVE/Pool ISA has no divide ALU op (walrus codegen rejects
# it), so 0 is the only working setting on hardware.
USE_DIV = os.environ.get("USE_DIV", "0") == "1"


def build_nc() -> bass.Bass:
    nc = bacc.Bacc()

    x_d = nc.declare_dram_parameter("x", [C, HALF], F32, isOutput=False)
    y_d = nc.declare_dram_parameter("y", [C, M], F32, isOutput=False)
    v_d = nc.declare_dram_parameter("v", [128, NT], F32, isOutput=True)

    with tile.TileContext(nc) as tc:
        with (
            tc.tile_pool(name="io", bufs=1) as io,
            tc.tile_pool(name="consts", bufs=1) as consts,
            tc.tile_pool(name="stats", bufs=1) as stats,
        ):
            y_bf = io.tile([128, 2, M], BF16)
            x_sb = io.tile([128, 2, HALF], F32)
            x_bf = io.tile([128, 2, HALF], BF16)
            ny_b = io.tile([128, M], F32)  # |Yc_j| (or its inverse) broadcast

            ones_col = consts.tile([128, 1], F32)
            nc.vector.memset(ones_col, 1.0)
            ones_col_bf = consts.tile([128, 1], BF16)
            nc.vector.memset(ones_col_bf, 1.0)
            bc_inv256 = consts.tile([128, 128], BF16)  # rank-reduce+broadcast mu
            nc.vector.memset(bc_inv256, 1.0 / 256.0)
            bc_ones = consts.tile([128, 128], BF16)  # rank-reduce+broadcast qy
            nc.vector.memset(bc_ones, 1.0)
            ten_col = consts.tile([128, 1], F32)
            nc.vector.memset(ten_col, 10.0)
            c1001_col = consts.tile([128, 1], F32)
            nc.vector.memset(c1001_col, 1.001)

            sy16 = stats.tile([128, NT], F32)  # sum_c Y over own-half cols
            sx16 = stats.tile([128, NT], F32)  # sum_c X
            sxx16 = stats.tile([128, NT], F32)  # sum_c X^2
            nx2 = stats.tile([128, NT], F32)
            inv_nx = stats.tile([128, NT], F32)
            r16 = stats.tile([128, NT], F32)
            sumw16 = stats.tile([128, NT], F32)
            maxw16 = stats.tile([128, NT], F32)
            rs16 = stats.tile([128, NT], F32)
            v16 = stats.tile([128, NT], F32)
            t16 = stats.tile([128, NT], F32)

            y_v = y_d.rearrange("(k p) m -> p k m", p=128)
            x_v = x_d.rearrange("(k p) m -> p k m", p=128)

            with (
                tc.tile_pool(name="pre", bufs=2, space="PSUM") as pre,
                tc.tile_pool(name="scr", bufs=3) as scr,
            ):
                # ---- input DMAs, alternating the two HWDGE queues ------
                ystage = []
                for q in range(4):
                    st = scr.tile([128, 2, Q], F32, tag="stage")
                    eng = nc.sync
                    eng.dma_start(out=st[:, :, :], in_=y_v[:, :, q * Q : (q + 1) * Q])
                    ystage.append(st)
                nc.sync.dma_start(out=x_sb[:, :, 0:Q], in_=x_v[:, :, 0:Q])
                nc.sync.dma_start(out=x_sb[:, :, Q:HALF], in_=x_v[:, :, Q:HALF])

                def stat16(dst, src_sb, tiles, ones):
                    """dst[p, t] = sum_c src[c, (t-tiles[0])*128 + p]."""
                    ps = pre.tile([128, len(tiles)], F32, tag="pre")
                    for i, t in enumerate(tiles):
                        for k in range(2):
                            nc.tensor.matmul(
                                ps[:, i : i + 1],
                                lhsT=src_sb[:, k, i * 128 : (i + 1) * 128],
                                rhs=ones[:, :],
                                start=(k == 0),
                                stop=(k == 1),
                            )
                    nc.vector.tensor_copy(
                        dst[:, tiles[0] : tiles[0] + len(tiles)], ps[:, :]
                    )

                def conv_quarter(q):
                    nc.scalar.copy(
                        y_bf[:, :, q * Q : (q + 1) * Q], ystage[q][:, :, :]
                    )

                def center_quarter(q):
                    # mu[p, j] = sum_c y[c, j] / 256 for every partition p via
                    # a single rank-reduce+broadcast matmul, then in-place sub
                    ps = pre.tile([128, Q], F32, tag="pre")
                    for j in range(2):
                        for k in range(2):
                            nc.tensor.matmul(
                                ps[:, j * 512 : (j + 1) * 512],
                                lhsT=bc_inv256[:, :],
                                rhs=y_bf[
                                    :, k, q * Q + j * 512 : q * Q + (j + 1) * 512
                                ],
                                start=(k == 0),
                                stop=(k == 1),
                            )
                    for k in range(2):
                        nc.vector.tensor_sub(
                            y_bf[:, k, q * Q : (q + 1) * Q],
                            y_bf[:, k, q * Q : (q + 1) * Q],
                            ps[:, :],
                        )

                def sq_quarter(q, on_dve):
                    sq = scr.tile([128, 2, Q], BF16, tag="sq")
                    src = y_bf[:, :, q * Q : (q + 1) * Q]
                    if on_dve:
                        nc.vector.tensor_mul(sq[:, :, :], src, src)
                    else:
                        nc.scalar.activation(sq[:, :, :], src, AF.Square)
                    return sq

                def ny_quarter(q, sq):
                    # qy[p, j] = sum_c Yc[c, j]^2 broadcast via ones lhsT;
                    # ny = sqrt(qy)
                    ps = pre.tile([128, Q], F32, tag="pre")
                    for j in range(2):
                        for k in range(2):
                            nc.tensor.matmul(
                                ps[:, j * 512 : (j + 1) * 512],
                                lhsT=bc_ones[:, :],
                                rhs=sq[:, k, j * 512 : (j + 1) * 512],
                                start=(k == 0),
                                stop=(k == 1),
                            )
                    if USE_DIV:
                        nc.scalar.activation(
                            ny_b[:, q * Q : (q + 1) * Q], ps[:, :], AF.Sqrt
                        )
                    else:
                        t = scr.tile([128, Q], F32, tag="t")
                        nc.vector.reciprocal(t[:, :], ps[:, :])
                        nc.scalar.activation(
                            ny_b[:, q * Q : (q + 1) * Q], t[:, :], AF.Sqrt
                        )

                def x_side():
                    nc.scalar.copy(x_bf[:, :, 0:Q], x_sb[:, :, 0:Q])
                    nc.scalar.copy(x_bf[:, :, Q:HALF], x_sb[:, :, Q:HALF])
                    sqx = scr.tile([128, 2, HALF], BF16, tag="sqx")
                    nc.vector.tensor_mul(sqx[:, :, :], x_bf[:, :, :], x_bf[:, :, :])
                    stat16(sx16, x_sb, list(range(NT)), ones_col)
                    stat16(sxx16, sqx, list(range(NT)), ones_col_bf)
                    # nx2 = sxx - (sy/256)*(2*sx - sy)  (tiny DVE ops)
                    nc.vector.tensor_scalar(
                        out=t16[:, :], in0=sx16[:, :], scalar1=2.0, scalar2=None,
                        op0=ALU.mult,
                    )
                    nc.vector.tensor_sub(t16[:, :], t16[:, :], sy16[:, :])
                    nc.vector.tensor_mul(t16[:, :], t16[:, :], sy16[:, :])
                    nc.vector.tensor_scalar(
                        out=t16[:, :], in0=t16[:, :], scalar1=1.0 / 256.0,
                        scalar2=None, op0=ALU.mult,
                    )
                    nc.vector.tensor_sub(nx2[:, :], sxx16[:, :], t16[:, :])
                    nc.vector.reciprocal(t16[:, :], nx2[:, :])
                    nc.scalar.activation(inv_nx[:, :], t16[:, :], AF.Sqrt)  # 1/|Xc|

                # ---- phase schedule --------------------------------------
                for q in range(4):
                    if q < 2:
                        stat16(
                            sy16, ystage[q], list(range(q * 8, (q + 1) * 8)),
                            ones_col,
                        )
                    conv_quarter(q)
                    center_quarter(q)
                    sq = sq_quarter(q, on_dve=(q % 2 == 0))
                    ny_quarter(q, sq)
                    if q == 2:
                        x_side()

            # ---- main loop -------------------------------------------------
            with (
                tc.tile_pool(name="psum_g", bufs=2, space="PSUM") as psum_g,
                tc.tile_pool(name="dpool", bufs=2) as dpool,
                tc.tile_pool(name="wpool", bufs=1) as wpool,
                tc.tile_pool(name="mpool", bufs=2) as mpool,
                tc.tile_pool(name="mains", bufs=2) as mains,
            ):

                def half(t, h, d_sb):
                    ps = psum_g.tile([128, HB], F32, tag="g")
                    for j in range(4):
                        for k in range(2):
                            nc.tensor.matmul(
                                ps[:, j * 512 : (j + 1) * 512],
                                lhsT=x_bf[:, k, t * 128 : (t + 1) * 128],
                                rhs=y_bf[
                                    :, k, h * HB + j * 512 : h * HB + (j + 1) * 512
                                ],
                                start=(k == 0),
                                stop=(k == 1),
                            )
                    # fused eviction: d = (ps * 1/|Xc_i|) * 1/|Yc_j| = cos
                    nc.vector.scalar_tensor_tensor(
                        out=d_sb[:, h * HB : (h + 1) * HB],
                        in0=ps[:, :],
                        scalar=inv_nx[:, t : t + 1],
                        in1=ny_b[:, h * HB : (h + 1) * HB],
                        op0=ALU.mult,
                        op1=ALU.mult,
                    )

                def max_tree(t, d_sb):
                    # pairwise-max tree over bf16 cos (2x DVE mode), then a
                    # short reduce; ~2.6us vs 4.4us for straight reduces
                    m = mpool.tile([128, 3584], BF16, tag="m")
                    cmax = mains.tile([128, 1], F32, tag="cmax")
                    nc.vector.tensor_tensor(
                        out=m[:, 0:2048], in0=d_sb[:, 0:2048],
                        in1=d_sb[:, 2048:4096], op=ALU.max,
                    )
                    nc.vector.tensor_tensor(
                        out=m[:, 2048:3072], in0=m[:, 0:1024],
                        in1=m[:, 1024:2048], op=ALU.max,
                    )
                    nc.vector.tensor_tensor(
                        out=m[:, 3072:3584], in0=m[:, 2048:2560],
                        in1=m[:, 2560:3072], op=ALU.max,
                    )
                    nc.vector.reduce_max(
                        cmax, m[:, 3072:3584], axis=mybir.AxisListType.X
                    )
                    return cmax

                def smalls_part1(t, cmax):
                    # u = dmin + 1e-3 = 1.001 - cosmax
                    u = mains.tile([128, 1], F32, tag="u")
                    nc.scalar.activation(
                        u, cmax, AF.Identity, scale=-1.0, bias=c1001_col[:, :]
                    )
                    return u

                def smalls_part2(t):
                    scale_i = mains.tile([128, 1], F32, tag="scale")
                    bias_i = mains.tile([128, 1], F32, tag="bias")
                    nc.scalar.activation(
                        scale_i, r16[:, t : t + 1], AF.Identity, scale=10.0
                    )
                    nc.scalar.activation(
                        bias_i, r16[:, t : t + 1], AF.Identity,
                        scale=-10.0, bias=ten_col[:, :],
                    )
                    return scale_i, bias_i

                def exp_tile(t, d_sb, scale_i, bias_i):
                    w_sb = wpool.tile([128, M], BF16, tag="w")
                    nc.scalar.activation(
                        out=w_sb[:, :],
                        in_=d_sb[:, :],
                        func=AF.Exp,
                        bias=bias_i,
                        scale=scale_i,
                        accum_out=sumw16[:, t : t + 1],
                    )

                prev = None
                for t in range(NT):
                    d_sb = dpool.tile([128, M], BF16, tag="d")
                    half(t, 0, d_sb)
                    if prev is not None:
                        # previous tile's reciprocal rides between this tile's
                        # two evictions so DVE never stalls on the u chain
                        pt, pu, pd = prev
                        nc.vector.reciprocal(r16[:, pt : pt + 1], pu)
                        psc, pbi = smalls_part2(pt)
                        exp_tile(pt, pd, psc, pbi)
                    half(t, 1, d_sb)
                    cmax = max_tree(t, d_sb)
                    u = smalls_part1(t, cmax)
                    prev = (t, u, d_sb)

                pt, pu, pd = prev
                nc.vector.reciprocal(r16[:, pt : pt + 1], pu)
                psc, pbi = smalls_part2(pt)
                exp_tile(pt, pd, psc, pbi)

                # ---- epilogue: v = exp(0.01*r) / sumw ---------------------
                nc.scalar.activation(maxw16[:, :], r16[:, :], AF.Exp, scale=0.01)
                nc.vector.reciprocal(rs16[:, :], sumw16[:, :])
                nc.vector.tensor_mul(v16[:, :], maxw16[:, :], rs16[:, :])
                nc.sync.dma_start(out=v_d[:, :], in_=v16[:, :])

    nc.compile()
    return nc


_NC = None


def _get_nc():
    global _NC
    if _NC is None:
        _NC = build_nc()
    return _NC


def make_in_maps(X, Y):
    """Per-core inputs. Y columns permuted to [own-half | other-half]."""
    in_maps = []
    for c in range(N_CORES):
        b, h = c // 2, c % 2
        xs = np.ascontiguousarray(X[b][:, h * HALF : (h + 1) * HALF])
        ys = np.ascontiguousarray(
            np.concatenate(
                [
                    Y[b][:, h * HALF : (h + 1) * HALF],
                    Y[b][:, (1 - h) * HALF : (2 - h) * HALF],
                ],
                axis=1,
            )
        )
        in_maps.append({"x": xs, "y": ys})
    return in_maps


def finish_host(results):
    """results: list of 8 per-core dicts with 'v' [128, NT]."""
    cx = np.zeros(B, dtype=np.float64)
    for c in range(N_CORES):
        cx[c // 2] += results[c]["v"].astype(np.float64).sum()
    cx /= M
    return np.float32(np.mean(-np.log(cx)))


def run(X_features, Y_features, trace=False, tmpdir=None):
    X = np.asarray(X_features, dtype=np.float32).reshape(B, C, M)
    Y = np.asarray(Y_features, dtype=np.float32).reshape(B, C, M)
    nc = _get_nc()
    res = run_bass_kernel_spmd(
        nc, make_in_maps(X, Y), list(range(N_CORES)), trace=trace, tmpdir=tmpdir
    )
    return finish_host(res.results), res


def kernel(X_features, Y_features):
    loss, _ = run(X_features, Y_features)
    return loss


# revision 49
# speedup vs baseline: 1.3029x; 1.3029x over previous
"""ContextualLoss forward on 8 Trainium2 NeuronCores.

Math (reference):
    mu[m]   = mean_c Y[c, m]                      (PONO over channels of Y)
    Xc = X - mu ; Yc = Y - mu                     (both centered by Y's mean)
    cos[i,j] = <Xc_i, Yc_j> / (|Xc_i| |Yc_j|)
    d = 1 - cos ; dn = d / (min_j d + 1e-3) ; w = exp((1 - dn)/0.1)
    A = w / sum_j w ; CX_b = mean_i max_j A ; loss = mean_b -log CX_b

Device-side structure (per core: one sample b, one 2048-row half):
  * Only Y is centered. <Xc_i, Yc_j> == <X_i, Yc_j> because Yc has zero
    channel-mean, so raw X feeds the matmul.
  * |Xc_i|^2 = sum X^2 - mu*(2*sum X - 256*mu) via cheap N=1 column-sum
    matmuls -- Xc is never materialized.
  * Main loop per 128-row tile: 16 bf16 matmuls -> PSUM halves [128,2048];
    one fused DVE tensor_tensor_reduce per half does the PSUM->SBUF move,
    the 1/|Yc_j| column scale (op0=divide by |Yc_j|), and the running
    row-max in a single pass. DVE runs nothing else in steady state.
  * Per-tile scalar chain (dmin -> exp scale/bias) runs on the Pool engine.
  * ScalarE does one fused Exp per tile; accum_out gives sum_j w for free.
  * max_j A = exp(0.01/(dmin+1e-3)) / sum_j w  (w monotone in d).

Sharding: core c -> sample b = c//2, row-half h = c%2 (2048 rows each).
Each core's Y is column-permuted host-side to [own-half | other-half] so the
identical SPMD program can read the X-half's means from columns [0, 2048).
Row reductions are permutation-invariant, so the permutation is harmless.
"""

import os
import sys

sys.path.insert(0, "/opt/trn_rl_repo")

import numpy as np

import concourse.bass as bass
import concourse.tile as tile
from concourse import bacc
from concourse import mybir
from concourse.bass_utils import run_bass_kernel_spmd

B = 4
C = 256
M = 4096  # 64*64 spatial positions
HALF = M // 2  # rows per core
NT = HALF // 128  # 16 i-tiles per core
N_CORES = 8
Q = 1024  # preprocessing quarter width
HB = 2048  # main-loop PSUM half width

F32 = mybir.dt.float32
BF16 = mybir.dt.bfloat16
AF = mybir.ActivationFunctionType
ALU = mybir.AluOpType

NEG_HUGE = -3.0e38

# 1: TTR divides PSUM scores by |Yc_j| directly (no reciprocal pass).
# 0: precompute 1/|Yc_j| (DVE reciprocal) and multiply in the TTR.
# NOTE: the real DVE/Pool ISA has no divide ALU op (walrus codegen rejects
# it), so 0 is the only working setting on hardware.
USE_DIV = os.environ.get("USE_DIV", "0") == "1"


def build_nc() -> bass.Bass:
    nc = bacc.Bacc()

    x_d = nc.declare_dram_parameter("x", [C, HALF], F32, isOutput=False)
    y_d = nc.declare_dram_parameter("y", [C, M], F32, isOutput=False)
    v_d = nc.declare_dram_parameter("v", [128, NT], F32, isOutput=True)

    with tile.TileContext(nc) as tc:
        with (
            tc.tile_pool(name="io", bufs=1) as io,
            tc.tile_pool(name="consts", bufs=1) as consts,
            tc.tile_pool(name="stats", bufs=1) as stats,
            tc.tile_pool(name="psum_ka", bufs=1, space="PSUM") as psum_ka,
        ):
            # per-quarter tiles keep the Tile dependency tracking (which is
            # per-tile, not per-range) from serializing the quarter chains
            y_q = [
                io.tile([128, 2, Q], BF16, name=f"y_q{i}") for i in range(4)
            ]
            x_sb = io.tile([128, 2, HALF], F32)
            x_bf = io.tile([128, 2, HALF], BF16)
            # 1/|Yc_j| broadcast, per quarter
            ny_q = [io.tile([128, Q], F32, name=f"ny_q{i}") for i in range(4)]

            ones_col = consts.tile([128, 1], F32)
            nc.vector.memset(ones_col, 1.0)
            ones_col_bf = consts.tile([128, 1], BF16)
            nc.vector.memset(ones_col_bf, 1.0)
            bc_inv256 = consts.tile([128, 128], BF16)  # rank-reduce+broadcast mu
            nc.vector.memset(bc_inv256, 1.0 / 256.0)
            bc_ones = consts.tile([128, 128], BF16)  # rank-reduce+broadcast qy
            nc.vector.memset(bc_ones, 1.0)
            ten_col = consts.tile([128, 1], F32)
            nc.vector.memset(ten_col, 10.0)
            c1001_col = consts.tile([128, 1], F32)
            nc.vector.memset(c1001_col, 1.001)
            ones_512 = consts.tile([128, 512], BF16)
            nc.vector.memset(ones_512, 1.0)

            sy16 = stats.tile([128, NT], F32)  # sum_c Y over own-half cols
            sx16 = stats.tile([128, NT], F32)  # sum_c X
            sxx16 = stats.tile([128, NT], F32)  # sum_c X^2
            nx2 = stats.tile([128, NT], F32)
            inv_nx = stats.tile([128, NT], F32)
            r16 = stats.tile([128, NT], F32)
            sumwA = stats.tile([128, NT], F32)
            sumwB = stats.tile([128, NT], F32)
            maxw16 = stats.tile([128, NT], F32)
            rs16 = stats.tile([128, NT], F32)
            v16 = stats.tile([128, NT], F32)
            t16 = stats.tile([128, NT], F32)

            y_v = y_d.rearrange("(k p) m -> p k m", p=128)
            x_v = x_d.rearrange("(k p) m -> p k m", p=128)

            # keep-alive PSUM bank: dependency-free filler matmuls park here
            # so the PE p-state never down-clocks between real bursts
            ka_ps = psum_ka.tile([128, 512], F32)

            def ka(n):
                for _ in range(n):
                    nc.tensor.matmul(
                        ka_ps[:, :], lhsT=bc_ones[:, :], rhs=ones_512[:, :],
                        start=True, stop=True,
                    )

            with (
                tc.tile_pool(name="pre", bufs=2, space="PSUM") as pre,
                tc.tile_pool(name="stg", bufs=4) as stg,
                tc.tile_pool(name="scr", bufs=3) as scr,
            ):
                # ---- input DMAs: y splits across both HWDGE queues, x
                # rides the Pool SWDGE queue -> everything lands by ~6.5us
                ystage = []
                for q in range(4):
                    st = stg.tile([128, 2, Q], F32, tag="stage")
                    eng = nc.sync if q % 2 == 0 else nc.scalar
                    eng.dma_start(out=st[:, :, :], in_=y_v[:, :, q * Q : (q + 1) * Q])
                    ystage.append(st)
                nc.gpsimd.dma_start(out=x_sb[:, :, 0:Q], in_=x_v[:, :, 0:Q])
                nc.gpsimd.dma_start(out=x_sb[:, :, Q:HALF], in_=x_v[:, :, Q:HALF])

                def stat16(dst, src_sb, tiles, ones):
                    """dst[p, t] = sum_c src[c, (t-tiles[0])*128 + p]."""
                    ps = pre.tile([128, len(tiles)], F32, tag="pre")
                    for i, t in enumerate(tiles):
                        for k in range(2):
                            nc.tensor.matmul(
                                ps[:, i : i + 1],
                                lhsT=src_sb[:, k, i * 128 : (i + 1) * 128],
                                rhs=ones[:, :],
                                start=(k == 0),
                                stop=(k == 1),
                            )
                    nc.vector.tensor_copy(
                        dst[:, tiles[0] : tiles[0] + len(tiles)], ps[:, :]
                    )

                def conv_quarter(q):
                    nc.scalar.copy(y_q[q][:, :, :], ystage[q][:, :, :])

                def center_mm(q):
                    # mu[p, j] = sum_c y[c, j] / 256 for every partition p via
                    # a single rank-reduce+broadcast matmul
                    ps = pre.tile([128, Q], F32, tag="pre")
                    for j in range(2):
                        for k in range(2):
                            nc.tensor.matmul(
                                ps[:, j * 512 : (j + 1) * 512],
                                lhsT=bc_inv256[:, :],
                                rhs=y_q[q][:, k, j * 512 : (j + 1) * 512],
                                start=(k == 0),
                                stop=(k == 1),
                            )
                    return ps

                def center_sub(q, ps):
                    for k in range(2):
                        nc.vector.tensor_sub(
                            y_q[q][:, k, :], y_q[q][:, k, :], ps[:, :]
                        )

                def sq_quarter(q, on_dve):
                    sq = scr.tile([128, 2, Q], BF16, tag="sq")
                    src = y_q[q][:, :, :]
                    if on_dve:
                        nc.vector.tensor_mul(sq[:, :, :], src, src)
                    else:
                        nc.scalar.activation(sq[:, :, :], src, AF.Square)
                    return sq

                def ny_quarter(q, sq):
                    # qy[p, j] = sum_c Yc[c, j]^2 broadcast via ones lhsT;
                    # inv_ny = sqrt(1/qy)
                    ps = pre.tile([128, Q], F32, tag="pre")
                    for j in range(2):
                        for k in range(2):
                            nc.tensor.matmul(
                                ps[:, j * 512 : (j + 1) * 512],
                                lhsT=bc_ones[:, :],
                                rhs=sq[:, k, j * 512 : (j + 1) * 512],
                                start=(k == 0),
                                stop=(k == 1),
                            )
                    t = scr.tile([128, Q], F32, tag="t")
                    nc.vector.reciprocal(t[:, :], ps[:, :])
                    nc.scalar.activation(ny_q[q][:, :], t[:, :], AF.Sqrt)

                # ---- phase schedule. Engine queues are in-order and the
                # cross-engine semaphores are cumulative counters, so each
                # queue is laid out by readiness time:
                #   ScalarE: conv0-3, sq1, xa, sq3, xb, sqrt0-3, invnx
                #   DVE: sub0-3, sq0, sq2, recip0-3, sqx, nx2 chain
                #   PE: ka, cmm0-3 (ka-bridged), stats, nymm0-3
                ka(10)  # ramp the PE clock while DMAs land
                conv_quarter(0)
                conv_quarter(1)
                conv_quarter(2)
                conv_quarter(3)
                cps = [None] * 4
                cps[0] = center_mm(0)
                ka(4)
                center_sub(0, cps[0])
                cps[1] = center_mm(1)
                ka(4)
                center_sub(1, cps[1])
                cps[2] = center_mm(2)
                ka(4)
                center_sub(2, cps[2])
                cps[3] = center_mm(3)
                center_sub(3, cps[3])
                stat16(sy16, ystage[0], list(range(0, 8)), ones_col)
                stat16(sy16, ystage[1], list(range(8, 16)), ones_col)
                sq0 = sq_quarter(0, on_dve=True)
                sq1 = sq_quarter(1, on_dve=False)
                nc.scalar.copy(x_bf[:, :, 0:Q], x_sb[:, :, 0:Q])
                sq2 = sq_quarter(2, on_dve=True)
                sq3 = sq_quarter(3, on_dve=False)
                nc.scalar.copy(x_bf[:, :, Q:HALF], x_sb[:, :, Q:HALF])
                ny_quarter(0, sq0)
                ny_quarter(1, sq1)
                ny_quarter(2, sq2)
                ny_quarter(3, sq3)
                ka(4)
                sqx = scr.tile([128, 2, HALF], BF16, tag="sqx")
                nc.vector.tensor_mul(sqx[:, :, :], x_bf[:, :, :], x_bf[:, :, :])
                stat16(sx16, x_sb, list(range(NT)), ones_col)
                stat16(sxx16, sqx, list(range(NT)), ones_col_bf)
                # nx2 = sxx - (sy/256)*(2*sx - sy)  (tiny DVE ops)
                nc.vector.tensor_scalar(
                    out=t16[:, :], in0=sx16[:, :], scalar1=2.0, scalar2=None,
                    op0=ALU.mult,
                )
                nc.vector.tensor_sub(t16[:, :], t16[:, :], sy16[:, :])
                nc.vector.tensor_mul(t16[:, :], t16[:, :], sy16[:, :])
                nc.vector.tensor_scalar(
                    out=t16[:, :], in0=t16[:, :], scalar1=1.0 / 256.0,
                    scalar2=None, op0=ALU.mult,
                )
                nc.vector.tensor_sub(nx2[:, :], sxx16[:, :], t16[:, :])
                nc.vector.reciprocal(t16[:, :], nx2[:, :])
                nc.scalar.activation(inv_nx[:, :], t16[:, :], AF.Sqrt)  # 1/|Xc|

            # ---- main loop -------------------------------------------------
            with (
                tc.tile_pool(name="psum_g", bufs=3, space="PSUM") as psum_g,
                tc.tile_pool(name="dpool", bufs=3) as dpool,
                tc.tile_pool(name="wpool", bufs=1) as wpool,
                tc.tile_pool(name="mpool", bufs=2) as mpool,
                tc.tile_pool(name="mains", bufs=2) as mains,
            ):

                def quarter_mm(t, g):
                    ps = psum_g.tile([128, Q], F32, tag="g")
                    for k in range(2):
                        for j in range(2):
                            nc.tensor.matmul(
                                ps[:, j * 512 : (j + 1) * 512],
                                lhsT=x_bf[:, k, t * 128 : (t + 1) * 128],
                                rhs=y_q[g][:, k, j * 512 : (j + 1) * 512],
                                start=(k == 0),
                                stop=(k == 1),
                            )
                    return ps

                def evict_dve(t, g, ps, d_sb):
                    # fused eviction: d = (ps * 1/|Xc_i|) * 1/|Yc_j| = cos
                    nc.vector.scalar_tensor_tensor(
                        out=d_sb[:, g * Q : (g + 1) * Q],
                        in0=ps[:, :],
                        scalar=inv_nx[:, t : t + 1],
                        in1=ny_q[g][:, :],
                        op0=ALU.mult,
                        op1=ALU.mult,
                    )

                def evict_scalar(t, g, ps, d_sb):
                    # quarter g's Y columns carry 1/|Yc_j| already; ScalarE
                    # applies the per-row 1/|Xc_i| during the PSUM read
                    nc.scalar.activation(
                        d_sb[:, g * Q : (g + 1) * Q], ps[:, :], AF.Identity,
                        scale=inv_nx[:, t : t + 1],
                    )

                def max_tree(t, d_sb):
                    # pairwise-max tree over bf16 cos (2x DVE mode), then a
                    # short reduce; ~2.6us vs 4.4us for straight reduces
                    m = mpool.tile([128, 3584], BF16, tag="m")
                    cmax = mains.tile([128, 1], F32, tag="cmax")
                    nc.vector.tensor_tensor(
                        out=m[:, 0:2048], in0=d_sb[:, 0:2048],
                        in1=d_sb[:, 2048:4096], op=ALU.max,
                    )
                    nc.vector.tensor_tensor(
                        out=m[:, 2048:3072], in0=m[:, 0:1024],
                        in1=m[:, 1024:2048], op=ALU.max,
                    )
                    nc.vector.tensor_tensor(
                        out=m[:, 3072:3584], in0=m[:, 2048:2560],
                        in1=m[:, 2560:3072], op=ALU.max,
                    )
                    nc.vector.reduce_max(
                        cmax, m[:, 3072:3584], axis=mybir.AxisListType.X
                    )
                    return cmax

                def smalls_part1(t, cmax):
                    # u = dmin + 1e-3 = 1.001 - cosmax
                    u = mains.tile([128, 1], F32, tag="u")
                    nc.scalar.activation(
                        u, cmax, AF.Identity, scale=-1.0, bias=c1001_col[:, :]
                    )
                    return u

                def smalls_part2(t):
                    scale_i = mains.tile([128, 1], F32, tag="scale")
                    bias_i = mains.tile([128, 1], F32, tag="bias")
                    nc.scalar.activation(
                        scale_i, r16[:, t : t + 1], AF.Identity, scale=10.0
                    )
                    nc.scalar.activation(
                        bias_i, r16[:, t : t + 1], AF.Identity,
                        scale=-10.0, bias=ten_col[:, :],
                    )
                    return scale_i, bias_i

                def exp_tile(t, d_sb, scale_i, bias_i):
                    w_sb = wpool.tile([128, M], BF16, tag="w")
                    nc.scalar.activation(
                        out=w_sb[:, :],
                        in_=d_sb[:, :],
                        func=AF.Exp,
                        bias=bias_i,
                        scale=scale_i,
                        accum_out=sumwA[:, t : t + 1],
                    )

                ps_pend = [quarter_mm(0, g) for g in range(3)]
                prev = None
                for t in range(NT):
                    d_sb = dpool.tile([128, M], BF16, tag="d")
                    ps0, ps1, ps2 = ps_pend
                    ka(5)
                    ps3 = quarter_mm(t, 3)
                    if prev is not None:
                        # previous tile's reciprocal leads the DVE queue (its
                        # input u was finished last cycle -> no stall)
                        pt, pu, pd = prev
                        nc.vector.reciprocal(r16[:, pt : pt + 1], pu)
                        psc, pbi = smalls_part2(pt)
                    evict_dve(t, 0, ps0, d_sb)
                    evict_dve(t, 1, ps1, d_sb)
                    if prev is not None:
                        exp_tile(pt, pd, psc, pbi)
                    evict_dve(t, 2, ps2, d_sb)
                    evict_dve(t, 3, ps3, d_sb)
                    if t + 1 < NT:
                        ps_pend = [quarter_mm(t + 1, g) for g in range(3)]
                    cmax = max_tree(t, d_sb)
                    u = smalls_part1(t, cmax)
                    prev = (t, u, d_sb)

                pt, pu, pd = prev
                nc.vector.reciprocal(r16[:, pt : pt + 1], pu)
                psc, pbi = smalls_part2(pt)
                exp_tile(pt, pd, psc, pbi)

                # ---- epilogue: v = exp(0.01*r) / sumw ---------------------
                nc.scalar.activation(maxw16[:, :], r16[:, :], AF.Exp, scale=0.01)
                nc.vector.reciprocal(rs16[:, :], sumwA[:, :])
                nc.vector.tensor_mul(v16[:, :], maxw16[:, :], rs16[:, :])
                nc.sync.dma_start(out=v_d[:, :], in_=v16[:, :])

    nc.compile()
    return nc


_NC = None


def _get_nc():
    global _NC
    if _NC is None:
        _NC = build_nc()
    return _NC


def make_in_maps(X, Y):
    """Per-core inputs. Y columns permuted to [own-half | other-half]."""
    in_maps = []
    for c in range(N_CORES):
        b, h = c // 2, c % 2
        xs = np.ascontiguousarray(X[b][:, h * HALF : (h + 1) * HALF])
        ys = np.ascontiguousarray(
            np.concatenate(
                [
                    Y[b][:, h * HALF : (h + 1) * HALF],
                    Y[b][:, (1 - h) * HALF : (2 - h) * HALF],
                ],
                axis=1,
            )
        )
        in_maps.append({"x": xs, "y": ys})
    return in_maps


def finish_host(results):
    """results: list of 8 per-core dicts with 'v' [128, NT]."""
    cx = np.zeros(B, dtype=np.float64)
    for c in range(N_CORES):
        cx[c // 2] += results[c]["v"].astype(np.float64).sum()
    cx /= M
    return np.float32(np.mean(-np.log(cx)))


def run(X_features, Y_features, trace=False, tmpdir=None):
    X = np.asarray(X_features, dtype=np.float32).reshape(B, C, M)
    Y = np.asarray(Y_features, dtype=np.float32).reshape(B, C, M)
    nc = _get_nc()
    res = run_bass_kernel_spmd(
        nc, make_in_maps(X, Y), list(range(N_CORES)), trace=trace, tmpdir=tmpdir
    )
    return finish_host(res.results), res


def kernel(X_features, Y_features):
    loss, _ = run(X_features, Y_features)
    return loss


# revision 50
# speedup vs baseline: 1.4484x; 1.1117x over previous
"""ContextualLoss forward on 8 Trainium2 NeuronCores.

Math (reference):
    mu[m]   = mean_c Y[c, m]                      (PONO over channels of Y)
    Xc = X - mu ; Yc = Y - mu                     (both centered by Y's mean)
    cos[i,j] = <Xc_i, Yc_j> / (|Xc_i| |Yc_j|)
    d = 1 - cos ; dn = d / (min_j d + 1e-3) ; w = exp((1 - dn)/0.1)
    A = w / sum_j w ; CX_b = mean_i max_j A ; loss = mean_b -log CX_b

Device-side structure (per core: one sample b, one 2048-row half):
  * Only Y is centered. <Xc_i, Yc_j> == <X_i, Yc_j> because Yc has zero
    channel-mean, so raw X feeds the matmul.
  * |Xc_i|^2 = sum X^2 - mu*(2*sum X - 256*mu) via cheap N=1 column-sum
    matmuls -- Xc is never materialized.
  * Main loop per 128-row tile: 16 bf16 matmuls -> PSUM halves [128,2048];
    one fused DVE tensor_tensor_reduce per half does the PSUM->SBUF move,
    the 1/|Yc_j| column scale (op0=divide by |Yc_j|), and the running
    row-max in a single pass. DVE runs nothing else in steady state.
  * Per-tile scalar chain (dmin -> exp scale/bias) runs on the Pool engine.
  * ScalarE does one fused Exp per tile; accum_out gives sum_j w for free.
  * max_j A = exp(0.01/(dmin+1e-3)) / sum_j w  (w monotone in d).

Sharding: core c -> sample b = c//2, row-half h = c%2 (2048 rows each).
Each core's Y is column-permuted host-side to [own-half | other-half] so the
identical SPMD program can read the X-half's means from columns [0, 2048).
Row reductions are permutation-invariant, so the permutation is harmless.
"""

import os
import sys

sys.path.insert(0, "/opt/trn_rl_repo")

import numpy as np

import concourse.bass as bass
import concourse.tile as tile
from concourse import bacc
from concourse import mybir
from concourse.bass_utils import run_bass_kernel_spmd

B = 4
C = 256
M = 4096  # 64*64 spatial positions
HALF = M // 2  # rows per core
NT = HALF // 128  # 16 i-tiles per core
N_CORES = 8
Q = 1024  # preprocessing quarter width
HB = 2048  # main-loop PSUM half width

F32 = mybir.dt.float32
BF16 = mybir.dt.bfloat16
AF = mybir.ActivationFunctionType
ALU = mybir.AluOpType

NEG_HUGE = -3.0e38

# 1: TTR divides PSUM scores by |Yc_j| directly (no reciprocal pass).
# 0: precompute 1/|Yc_j| (DVE reciprocal) and multiply in the TTR.
# NOTE: the real DVE/Pool ISA has no divide ALU op (walrus codegen rejects
# it), so 0 is the only working setting on hardware.
USE_DIV = os.environ.get("USE_DIV", "0") == "1"


def build_nc() -> bass.Bass:
    nc = bacc.Bacc()

    x_d = nc.declare_dram_parameter("x", [C, HALF], F32, isOutput=False)
    y_d = nc.declare_dram_parameter("y", [C, M], F32, isOutput=False)
    v_d = nc.declare_dram_parameter("v", [128, NT], F32, isOutput=True)

    with tile.TileContext(nc) as tc:
        with (
            tc.tile_pool(name="io", bufs=1) as io,
            tc.tile_pool(name="consts", bufs=1) as consts,
            tc.tile_pool(name="stats", bufs=1) as stats,
            tc.tile_pool(name="psum_ka", bufs=1, space="PSUM") as psum_ka,
        ):
            # per-quarter tiles keep the Tile dependency tracking (which is
            # per-tile, not per-range) from serializing the quarter chains
            y_q = [
                io.tile([128, 2, Q], BF16, name=f"y_q{i}") for i in range(4)
            ]
            x_sb = io.tile([128, 2, HALF], F32)
            x_bf = io.tile([128, 2, HALF], BF16)
            # 1/|Yc_j| broadcast, per quarter
            ny_q = [io.tile([128, Q], F32, name=f"ny_q{i}") for i in range(4)]

            ones_col = consts.tile([128, 1], F32)
            nc.vector.memset(ones_col, 1.0)
            ones_col_bf = consts.tile([128, 1], BF16)
            nc.vector.memset(ones_col_bf, 1.0)
            bc_inv256 = consts.tile([128, 128], BF16)  # rank-reduce+broadcast mu
            nc.vector.memset(bc_inv256, 1.0 / 256.0)
            bc_ones = consts.tile([128, 128], BF16)  # rank-reduce+broadcast qy
            nc.vector.memset(bc_ones, 1.0)
            ten_col = consts.tile([128, 1], F32)
            nc.vector.memset(ten_col, 10.0)
            c1001_col = consts.tile([128, 1], F32)
            nc.vector.memset(c1001_col, 1.001)
            ones_512 = consts.tile([128, 512], BF16)
            nc.vector.memset(ones_512, 1.0)

            sy16 = stats.tile([128, NT], F32)  # sum_c Y over own-half cols
            sx16 = stats.tile([128, NT], F32)  # sum_c X
            sxx16 = stats.tile([128, NT], F32)  # sum_c X^2
            nx2 = stats.tile([128, NT], F32)
            inv_nx = stats.tile([128, NT], F32)
            r16 = stats.tile([128, NT], F32)
            sumwA = stats.tile([128, NT], F32)
            sumwB = stats.tile([128, NT], F32)
            maxw16 = stats.tile([128, NT], F32)
            rs16 = stats.tile([128, NT], F32)
            v16 = stats.tile([128, NT], F32)
            t16 = stats.tile([128, NT], F32)

            y_v = y_d.rearrange("(k p) m -> p k m", p=128)
            x_v = x_d.rearrange("(k p) m -> p k m", p=128)

            # keep-alive PSUM bank: dependency-free filler matmuls park here
            # so the PE p-state never down-clocks between real bursts
            ka_ps = psum_ka.tile([128, 512], F32)

            def ka(n):
                for _ in range(n):
                    nc.tensor.matmul(
                        ka_ps[:, :], lhsT=bc_ones[:, :], rhs=ones_512[:, :],
                        start=True, stop=True,
                    )

            with (
                tc.tile_pool(name="pre", bufs=2, space="PSUM") as pre,
                tc.tile_pool(name="stg", bufs=4) as stg,
                tc.tile_pool(name="scr", bufs=3) as scr,
            ):
                # ---- input DMAs: y splits across both HWDGE queues, x
                # rides the Pool SWDGE queue -> everything lands by ~6.5us
                ystage = []
                for q in range(4):
                    st = stg.tile([128, 2, Q], F32, tag="stage")
                    eng = nc.sync if q % 2 == 0 else nc.scalar
                    eng.dma_start(out=st[:, :, :], in_=y_v[:, :, q * Q : (q + 1) * Q])
                    ystage.append(st)
                nc.gpsimd.dma_start(out=x_sb[:, :, 0:Q], in_=x_v[:, :, 0:Q])
                nc.gpsimd.dma_start(out=x_sb[:, :, Q:HALF], in_=x_v[:, :, Q:HALF])

                def stat16(dst, src_sb, tiles, ones):
                    """dst[p, t] = sum_c src[c, (t-tiles[0])*128 + p]."""
                    ps = pre.tile([128, len(tiles)], F32, tag="pre")
                    for i, t in enumerate(tiles):
                        for k in range(2):
                            nc.tensor.matmul(
                                ps[:, i : i + 1],
                                lhsT=src_sb[:, k, i * 128 : (i + 1) * 128],
                                rhs=ones[:, :],
                                start=(k == 0),
                                stop=(k == 1),
                            )
                    nc.vector.tensor_copy(
                        dst[:, tiles[0] : tiles[0] + len(tiles)], ps[:, :]
                    )

                def conv_quarter(q):
                    nc.scalar.copy(y_q[q][:, :, :], ystage[q][:, :, :])

                def center_mm(q):
                    # mu[p, j] = sum_c y[c, j] / 256 for every partition p via
                    # a single rank-reduce+broadcast matmul
                    ps = pre.tile([128, Q], F32, tag="pre")
                    for j in range(2):
                        for k in range(2):
                            nc.tensor.matmul(
                                ps[:, j * 512 : (j + 1) * 512],
                                lhsT=bc_inv256[:, :],
                                rhs=y_q[q][:, k, j * 512 : (j + 1) * 512],
                                start=(k == 0),
                                stop=(k == 1),
                            )
                    return ps

                def center_sub(q, ps):
                    for k in range(2):
                        nc.vector.tensor_sub(
                            y_q[q][:, k, :], y_q[q][:, k, :], ps[:, :]
                        )

                def sq_quarter(q, on_dve):
                    sq = scr.tile([128, 2, Q], BF16, tag="sq")
                    src = y_q[q][:, :, :]
                    if on_dve:
                        nc.vector.tensor_mul(sq[:, :, :], src, src)
                    else:
                        nc.scalar.activation(sq[:, :, :], src, AF.Square)
                    return sq

                def ny_quarter(q, sq):
                    # qy[p, j] = sum_c Yc[c, j]^2 broadcast via ones lhsT;
                    # inv_ny = sqrt(1/qy)
                    ps = pre.tile([128, Q], F32, tag="pre")
                    for j in range(2):
                        for k in range(2):
                            nc.tensor.matmul(
                                ps[:, j * 512 : (j + 1) * 512],
                                lhsT=bc_ones[:, :],
                                rhs=sq[:, k, j * 512 : (j + 1) * 512],
                                start=(k == 0),
                                stop=(k == 1),
                            )
                    t = scr.tile([128, Q], F32, tag="t")
                    nc.vector.reciprocal(t[:, :], ps[:, :])
                    nc.scalar.activation(ny_q[q][:, :], t[:, :], AF.Sqrt)
                    if q == 3:
                        # quarter 3 is evicted by ScalarE in the main loop,
                        # which can only apply per-row scales -> fold the
                        # per-column 1/|Yc_j| into Y here
                        ny_bf = scr.tile([128, Q], BF16, tag="nybf")
                        nc.scalar.copy(ny_bf[:, :], ny_q[q][:, :])
                        for k in range(2):
                            nc.vector.tensor_mul(
                                y_q[q][:, k, :], y_q[q][:, k, :], ny_bf[:, :]
                            )

                # ---- phase schedule. Engine queues are in-order and the
                # cross-engine semaphores are cumulative counters, so each
                # queue is laid out by readiness time:
                #   ScalarE: conv0-3, sq1, xa, sq3, xb, sqrt0-3, invnx
                #   DVE: sub0-3, sq0, sq2, recip0-3, sqx, nx2 chain
                #   PE: ka, cmm0-3 (ka-bridged), stats, nymm0-3
                ka(10)  # ramp the PE clock while DMAs land
                conv_quarter(0)
                conv_quarter(1)
                conv_quarter(2)
                conv_quarter(3)
                cps = [None] * 4
                cps[0] = center_mm(0)
                ka(4)
                center_sub(0, cps[0])
                cps[1] = center_mm(1)
                ka(4)
                center_sub(1, cps[1])
                cps[2] = center_mm(2)
                ka(4)
                center_sub(2, cps[2])
                cps[3] = center_mm(3)
                center_sub(3, cps[3])
                stat16(sy16, ystage[0], list(range(0, 8)), ones_col)
                stat16(sy16, ystage[1], list(range(8, 16)), ones_col)
                sq0 = sq_quarter(0, on_dve=True)
                sq1 = sq_quarter(1, on_dve=False)
                nc.scalar.copy(x_bf[:, :, 0:Q], x_sb[:, :, 0:Q])
                sq2 = sq_quarter(2, on_dve=True)
                sq3 = sq_quarter(3, on_dve=False)
                nc.scalar.copy(x_bf[:, :, Q:HALF], x_sb[:, :, Q:HALF])
                ny_quarter(0, sq0)
                ny_quarter(1, sq1)
                ny_quarter(2, sq2)
                ny_quarter(3, sq3)
                ka(4)
                sqx = scr.tile([128, 2, HALF], BF16, tag="sqx")
                nc.vector.tensor_mul(sqx[:, :, :], x_bf[:, :, :], x_bf[:, :, :])
                stat16(sx16, x_sb, list(range(NT)), ones_col)
                stat16(sxx16, sqx, list(range(NT)), ones_col_bf)
                # nx2 = sxx - (sy/256)*(2*sx - sy)  (tiny DVE ops)
                nc.vector.tensor_scalar(
                    out=t16[:, :], in0=sx16[:, :], scalar1=2.0, scalar2=None,
                    op0=ALU.mult,
                )
                nc.vector.tensor_sub(t16[:, :], t16[:, :], sy16[:, :])
                nc.vector.tensor_mul(t16[:, :], t16[:, :], sy16[:, :])
                nc.vector.tensor_scalar(
                    out=t16[:, :], in0=t16[:, :], scalar1=1.0 / 256.0,
                    scalar2=None, op0=ALU.mult,
                )
                nc.vector.tensor_sub(nx2[:, :], sxx16[:, :], t16[:, :])
                nc.vector.reciprocal(t16[:, :], nx2[:, :])
                nc.scalar.activation(inv_nx[:, :], t16[:, :], AF.Sqrt)  # 1/|Xc|

            # ---- main loop -------------------------------------------------
            with (
                tc.tile_pool(name="psum_g", bufs=3, space="PSUM") as psum_g,
                tc.tile_pool(name="dpool", bufs=3) as dpool,
                tc.tile_pool(name="wpool", bufs=1) as wpool,
                tc.tile_pool(name="mpool", bufs=2) as mpool,
                tc.tile_pool(name="mains", bufs=2) as mains,
            ):

                def quarter_mm(t, g):
                    ps = psum_g.tile([128, Q], F32, tag="g")
                    for k in range(2):
                        for j in range(2):
                            nc.tensor.matmul(
                                ps[:, j * 512 : (j + 1) * 512],
                                lhsT=x_bf[:, k, t * 128 : (t + 1) * 128],
                                rhs=y_q[g][:, k, j * 512 : (j + 1) * 512],
                                start=(k == 0),
                                stop=(k == 1),
                            )
                    return ps

                def evict_dve(t, g, ps, d_sb):
                    # fused eviction: d = (ps * 1/|Xc_i|) * 1/|Yc_j| = cos
                    nc.vector.scalar_tensor_tensor(
                        out=d_sb[:, g * Q : (g + 1) * Q],
                        in0=ps[:, :],
                        scalar=inv_nx[:, t : t + 1],
                        in1=ny_q[g][:, :],
                        op0=ALU.mult,
                        op1=ALU.mult,
                    )

                def evict_scalar(t, g, ps, d_sb):
                    # quarter g's Y columns carry 1/|Yc_j| already; ScalarE
                    # applies the per-row 1/|Xc_i| during the PSUM read
                    nc.scalar.activation(
                        d_sb[:, g * Q : (g + 1) * Q], ps[:, :], AF.Identity,
                        scale=inv_nx[:, t : t + 1],
                    )

                def max_tree(t, d_sb):
                    # pairwise-max tree over bf16 cos (2x DVE mode), then a
                    # short reduce; ~2.6us vs 4.4us for straight reduces
                    m = mpool.tile([128, 3584], BF16, tag="m")
                    cmax = mains.tile([128, 1], F32, tag="cmax")
                    nc.vector.tensor_tensor(
                        out=m[:, 0:2048], in0=d_sb[:, 0:2048],
                        in1=d_sb[:, 2048:4096], op=ALU.max,
                    )
                    nc.vector.tensor_tensor(
                        out=m[:, 2048:3072], in0=m[:, 0:1024],
                        in1=m[:, 1024:2048], op=ALU.max,
                    )
                    nc.vector.tensor_tensor(
                        out=m[:, 3072:3584], in0=m[:, 2048:2560],
                        in1=m[:, 2560:3072], op=ALU.max,
                    )
                    nc.vector.reduce_max(
                        cmax, m[:, 3072:3584], axis=mybir.AxisListType.X
                    )
                    return cmax

                def smalls_part1(t, cmax):
                    # u = dmin + 1e-3 = 1.001 - cosmax
                    u = mains.tile([128, 1], F32, tag="u")
                    nc.scalar.activation(
                        u, cmax, AF.Identity, scale=-1.0, bias=c1001_col[:, :]
                    )
                    return u

                def smalls_part2(t):
                    scale_i = mains.tile([128, 1], F32, tag="scale")
                    bias_i = mains.tile([128, 1], F32, tag="bias")
                    nc.scalar.activation(
                        scale_i, r16[:, t : t + 1], AF.Identity, scale=10.0
                    )
                    nc.scalar.activation(
                        bias_i, r16[:, t : t + 1], AF.Identity,
                        scale=-10.0, bias=ten_col[:, :],
                    )
                    return scale_i, bias_i

                def exp_tile(t, d_sb, scale_i, bias_i):
                    w_sb = wpool.tile([128, M], BF16, tag="w")
                    nc.scalar.activation(
                        out=w_sb[:, :],
                        in_=d_sb[:, :],
                        func=AF.Exp,
                        bias=bias_i,
                        scale=scale_i,
                        accum_out=sumwA[:, t : t + 1],
                    )

                ps_pend = [quarter_mm(0, g) for g in range(3)]
                prev = None
                for t in range(NT):
                    d_sb = dpool.tile([128, M], BF16, tag="d")
                    ps0, ps1, ps2 = ps_pend
                    ka(5)
                    ps3 = quarter_mm(t, 3)
                    if prev is not None:
                        # previous tile's reciprocal leads the DVE queue (its
                        # input u was finished last cycle -> no stall)
                        pt, pu, pd = prev
                        nc.vector.reciprocal(r16[:, pt : pt + 1], pu)
                        psc, pbi = smalls_part2(pt)
                    evict_dve(t, 0, ps0, d_sb)
                    evict_scalar(t, 3, ps3, d_sb)
                    evict_dve(t, 1, ps1, d_sb)
                    if prev is not None:
                        exp_tile(pt, pd, psc, pbi)
                    evict_dve(t, 2, ps2, d_sb)
                    if t + 1 < NT:
                        ps_pend = [quarter_mm(t + 1, g) for g in range(3)]
                    cmax = max_tree(t, d_sb)
                    u = smalls_part1(t, cmax)
                    prev = (t, u, d_sb)

                pt, pu, pd = prev
                nc.vector.reciprocal(r16[:, pt : pt + 1], pu)
                psc, pbi = smalls_part2(pt)
                exp_tile(pt, pd, psc, pbi)

                # ---- epilogue: v = exp(0.01*r) / sumw ---------------------
                nc.scalar.activation(maxw16[:, :], r16[:, :], AF.Exp, scale=0.01)
                nc.vector.reciprocal(rs16[:, :], sumwA[:, :])
                nc.vector.tensor_mul(v16[:, :], maxw16[:, :], rs16[:, :])
                nc.sync.dma_start(out=v_d[:, :], in_=v16[:, :])

    nc.compile()
    return nc


_NC = None


def _get_nc():
    global _NC
    if _NC is None:
        _NC = build_nc()
    return _NC


def make_in_maps(X, Y):
    """Per-core inputs. Y columns permuted to [own-half | other-half]."""
    in_maps = []
    for c in range(N_CORES):
        b, h = c // 2, c % 2
        xs = np.ascontiguousarray(X[b][:, h * HALF : (h + 1) * HALF])
        ys = np.ascontiguousarray(
            np.concatenate(
                [
                    Y[b][:, h * HALF : (h + 1) * HALF],
                    Y[b][:, (1 - h) * HALF : (2 - h) * HALF],
                ],
                axis=1,
            )
        )
        in_maps.append({"x": xs, "y": ys})
    return in_maps


def finish_host(results):
    """results: list of 8 per-core dicts with 'v' [128, NT]."""
    cx = np.zeros(B, dtype=np.float64)
    for c in range(N_CORES):
        cx[c // 2] += results[c]["v"].astype(np.float64).sum()
    cx /= M
    return np.float32(np.mean(-np.log(cx)))


def run(X_features, Y_features, trace=False, tmpdir=None):
    X = np.asarray(X_features, dtype=np.float32).reshape(B, C, M)
    Y = np.asarray(Y_features, dtype=np.float32).reshape(B, C, M)
    nc = _get_nc()
    res = run_bass_kernel_spmd(
        nc, make_in_maps(X, Y), list(range(N_CORES)), trace=trace, tmpdir=tmpdir
    )
    return finish_host(res.results), res


def kernel(X_features, Y_features):
    loss, _ = run(X_features, Y_features)
    return loss
